# revision 59
# baseline (speedup 1.0000x reference)
"""Trainium2 Bass kernel for nn_Attention_p_2757369004155.

Reference math (per p in 0..4):
  x [256, 1728] -> qkv = W_qkv @ x -> 8 heads of dim 32, N=1728
  attn = softmax((q*scale)^T k), out = v @ attn^T, y = W_p @ out + b

Sharding: 8 cores = 4 p-branches x 2 query-halves. Each core is fully
self-contained (K/V computed for all n, Q for its half). The host permutes
each core's n axis so its query half is always columns [0, 864) — softmax
and A@V are permutation-invariant over n, so only the core's own output
columns matter and those are in natural order.

Per-core program structure:
  - QKV projections with pre-transposed weights (q scale folded in on host).
    V^T is produced directly by using x as the stationary matmul operand,
    with a ones-column per head so A@V also yields softmax denominators.
  - QK^T runs in S^T orientation (ST[n, m] = sum_c K[c,n]Q[c,m]) as ONE
    Kc=96 stacked float32r matmul per head: kk = [Kr;Ke;Kr], qq =
    [Qr;Qr;Qe] (r = float32r-rounded, e = residual) gives
    Kr.Qr + Ke.Qr + Kr.Qe which is fp32-accurate (dropped Ke.Qe ~ 1e-8)
    at 1 cycle/row instead of fp32's 4.
  - A@V computes O^T[m, c] (exp(ST) as the stationary operand) so the fp32
    matmul streams only 33 output columns per step, and the softmax
    normalization becomes a per-partition scaled copy on the ScalarE.
  - O^T is transposed back with PE transpose-mode before the final
    projection + bias.
  - The second half of QKV (heads 4-7) and all V^T tiles are interleaved
    into head-group 0's attention loop, which is otherwise ACT(exp)-bound.
"""

import numpy as np

import concourse.bass as bass
import concourse.tile as tile
from concourse import bacc, mybir
from concourse.bass import ds
from concourse.bass_utils import run_bass_kernel_spmd
from concourse.masks import make_identity

F32 = mybir.dt.float32
F32R = mybir.dt.float32r
AF = mybir.ActivationFunctionType

N_CORES = 8
C = 256            # channels
NH = 8             # heads
HD = 32            # head dim
N = 1728           # sequence (12*12*12)
M = N // 2         # per-core query positions
MC = 432           # m chunk (PSUM bank = 512 fp32)
N_MC = M // MC     # 2 chunks
NT_SIZES = [128] * 13 + [64]          # n contraction tiles
MT_SIZES = [128] * 6 + [96]           # m tiles for the O^T matmuls
SCALE = HD ** -0.5


def build_program():
    nc = bacc.Bacc(
        "TRN2",
        target_bir_lowering=False,
        debug=False,
        enable_asserts=False,
        num_devices=N_CORES,
    )

    xf_d = nc.dram_tensor("xf", [C, N], F32, kind="ExternalInput").ap()
    wq_d = nc.dram_tensor("wqT", [C, C], F32, kind="ExternalInput").ap()
    wk_d = nc.dram_tensor("wkT", [C, C], F32, kind="ExternalInput").ap()
    wv_d = nc.dram_tensor("wvT", [C, C], F32, kind="ExternalInput").ap()
    wp_d = nc.dram_tensor("wpT", [C, C], F32, kind="ExternalInput").ap()
    b_d = nc.dram_tensor("bias", [C, 1], F32, kind="ExternalInput").ap()
    y_d = nc.dram_tensor("y", [C, M], F32, kind="ExternalOutput").ap()

    xf_r = xf_d.rearrange("(kt p) n -> p kt n", p=128)

    with tile.TileContext(nc) as tc:
        with (
            tc.tile_pool(name="persist", bufs=1) as sb,
            tc.tile_pool(name="rot", bufs=2) as rot,
            tc.tile_pool(name="stage", bufs=1) as stage,
        ):
            # ---- load inputs (k path first — it is the long pole) ----
            wk_sb = sb.tile([128, 2, 2, 128], F32, tag="wk")
            nc.sync.dma_start(out=wk_sb, in_=wk_d.rearrange("(kt p) (ot o) -> p kt ot o", p=128, o=128))
            xf_sb = sb.tile([128, 2, N], F32, tag="xf")
            wq_sb = sb.tile([128, 2, 2, 128], F32, tag="wq")
            nc.sync.dma_start(out=xf_sb[:, :, ds(0, MC)], in_=xf_r[:, :, ds(0, MC)])
            nc.sync.dma_start(out=wq_sb, in_=wq_d.rearrange("(kt p) (ot o) -> p kt ot o", p=128, o=128))
            for ch in range(1, 4):
                nc.sync.dma_start(out=xf_sb[:, :, ds(ch * MC, MC)], in_=xf_r[:, :, ds(ch * MC, MC)])
            wv_sb = sb.tile([128, 2, 256], F32, tag="wv")
            nc.gpsimd.dma_start(out=wv_sb, in_=wv_d.rearrange("(kt p) c -> p kt c", p=128))
            # wp/b/identity are only needed from the projection/transpose
            # phase; keep them off the front-critical DMA queues
            wp_sb = sb.tile([128, 2, 2, 128], F32, tag="wp")
            b_sb = sb.tile([128, 2, 1], F32, tag="b")
            ident = sb.tile([128, 128], F32, tag="ident")
            # trigger the ACT exp table load during the initial DMA window
            warm = sb.tile([128, 1], F32, tag="warm")
            nc.vector.memset(warm, 0.0)
            nc.scalar.activation(warm, warm, AF.Exp)

            # stacked rounded+residual float32r planes, one per head (Kc=96):
            # kk[:, h] = [Kr_h; Ke_h; Kr_h], qq[:, h] = [Qr_h; Qr_h; Qe_h]
            qq_sb = sb.tile([96, NH, M], F32R, tag="qq")
            kk_sb = sb.tile([96, NH, N], F32R, tag="kk")
            # V^T with a ones column per head: [n, head, 33] (33rd = ones)
            vt_sb = sb.tile([128, 14, NH, 33], F32, tag="vt")
            nc.vector.memset(vt_sb[:, :, :, 32:33], 1.0)
            on_t_sb = sb.tile([128, 7, C], F32, tag="on_t")   # O^T normalized [m, c]
            on_sb = sb.tile([128, 2, M], F32, tag="on")       # O [c, m]
            y_sb = sb.tile([128, 2, M], F32, tag="y")

            def emit_k_chunk(psum_pool, psum_tag, ot, nck, krg, keg):
                """k chunk for heads [4ot, 4ot+4): matmul + f32r split into staging."""
                pk = psum_pool.tile([128, MC], F32, tag=psum_tag, name="pk", bufs=2)
                for kt in range(2):
                    nc.tensor.matmul(
                        pk,
                        lhsT=wk_sb[:, kt, ot, :],
                        rhs=xf_sb[:, kt, ds(nck * MC, MC)],
                        start=(kt == 0),
                        stop=(kt == 1),
                    )
                sl = ds(nck * MC, MC)
                nc.vector.tensor_copy(krg[:, sl], pk)
                nc.vector.tensor_sub(keg[:, sl], pk, krg[:, sl].bitcast(F32))

            def emit_k_stacks(ot, lo, size, krg, keg, dual=False):
                for h in range(4):
                    gh = 4 * ot + h
                    hsl = ds(32 * h, 32)
                    sl = ds(lo, size)
                    eng = nc.sync if (not dual or h % 2 == 0) else nc.gpsimd
                    eng.dma_start(out=kk_sb[0:32, gh, sl], in_=krg[hsl, sl])
                    eng.dma_start(out=kk_sb[32:64, gh, sl], in_=keg[hsl, sl])
                    eng.dma_start(out=kk_sb[64:96, gh, sl], in_=krg[hsl, sl])

            def emit_q_mc(psum_pool, psum_tag, ot, mc, qrg, qeg):
                """q chunk (scale pre-folded into wqT): matmul + f32r split."""
                pq = psum_pool.tile([128, MC], F32, tag=psum_tag, name="pq", bufs=2)
                for kt in range(2):
                    nc.tensor.matmul(
                        pq,
                        lhsT=wq_sb[:, kt, ot, :],
                        rhs=xf_sb[:, kt, ds(mc * MC, MC)],
                        start=(kt == 0),
                        stop=(kt == 1),
                    )
                sl = ds(mc * MC, MC)
                nc.vector.tensor_copy(qrg[:, sl], pq)
                nc.vector.tensor_sub(qeg[:, sl], pq, qrg[:, sl].bitcast(F32))

            def emit_q_stacks(ot, qrg, qeg, lo=0, size=M, dual=False):
                for h in range(4):
                    gh = 4 * ot + h
                    hsl = ds(32 * h, 32)
                    sl = ds(lo, size)
                    eng = nc.sync if (not dual or h % 2 == 0) else nc.gpsimd
                    eng.dma_start(out=qq_sb[0:32, gh, sl], in_=qrg[hsl, sl])
                    eng.dma_start(out=qq_sb[32:64, gh, sl], in_=qrg[hsl, sl])
                    eng.dma_start(out=qq_sb[64:96, gh, sl], in_=qeg[hsl, sl])

            def emit_vt(psum_pool, nt, half):
                """vT[n, c] tile for one head group, x as the stationary operand."""
                w = NT_SIZES[nt]
                pv = psum_pool.tile([128, 128], F32, tag="aux", name="pv", bufs=2)
                for kt in range(2):
                    nc.tensor.matmul(
                        pv[:w, :],
                        lhsT=xf_sb[:, kt, ds(nt * 128, w)],
                        rhs=wv_sb[:, kt, ds(128 * half, 128)],
                        start=(kt == 0),
                        stop=(kt == 1),
                    )
                nc.vector.tensor_copy(
                    vt_sb[:w, nt, ds(4 * half, 4), 0:32],
                    pv[:w, :].rearrange("p (h c) -> p h c", h=4),
                )

            # ---- head-group 0's QKV ----
            with tc.tile_pool(name="psA", bufs=2, space="PSUM") as psA:
                # keep the PE p-state model warm while the first DMAs land
                wu_in = sb.tile([128, 128], F32, tag="wu_in")
                nc.vector.memset(wu_in, 0.0)
                for i in range(9):
                    wu = psA.tile([128, 128], F32, tag="wu", name="wu")
                    nc.tensor.matmul(wu, lhsT=wu_in, rhs=wu_in, start=True, stop=True)
                krg0 = stage.tile([128, N], F32R, tag="krg", name="krg0")
                keg0 = stage.tile([128, N], F32R, tag="keg", name="keg0")
                qrg0 = stage.tile([128, M], F32R, tag="qrg", name="qrg0")
                qeg0 = stage.tile([128, M], F32R, tag="qeg", name="qeg0")
                emit_k_chunk(psA, "pk", 0, 0, krg0, keg0)
                emit_k_stacks(0, 0, MC, krg0, keg0, dual=True)
                emit_q_mc(psA, "pk", 0, 0, qrg0, qeg0)
                emit_q_stacks(0, qrg0, qeg0, 0, MC, dual=True)
                emit_q_mc(psA, "pk", 0, 1, qrg0, qeg0)
                emit_q_stacks(0, qrg0, qeg0, MC, M - MC, dual=True)
                for nck in range(1, 4):
                    emit_k_chunk(psA, "pk", 0, nck, krg0, keg0)
                emit_k_stacks(0, MC, N - MC, krg0, keg0, dual=True)

            # late-needed loads, after the front-critical DMAs
            nc.gpsimd.dma_start(out=wp_sb, in_=wp_d.rearrange("(kt p) (ot o) -> p kt ot o", p=128, o=128))
            nc.gpsimd.dma_start(out=b_sb, in_=b_d.rearrange("(ot p) one -> p ot one", p=128))
            make_identity(nc, ident)

            # ---- attention (heads 4-7 QKV + vT interleaved into group 0) ----
            with tc.tile_pool(name="psB", bufs=1, space="PSUM") as psB:
                for g in range(2):
                    ot_ps = [
                        psB.tile([128, 7, 2, 36], F32, tag=f"ot{j}", name=f"ot{j}")
                        for j in range(2)
                    ]
                    def emit_st(nt, mc, ex):
                        """S^T chunk: one Kc=96 stacked matmul per head + exp."""
                        w = NT_SIZES[nt]
                        nsl = ds(nt * 128, w)
                        msl = ds(mc * MC, MC)
                        sts = [
                            psB.tile([128, 2, 512], F32, tag=f"st{j}", name=f"st{j}")
                            for j in range(2)
                        ]
                        for h in range(4):
                            nc.tensor.matmul(
                                sts[h // 2][:w, h % 2, 0:MC],
                                lhsT=kk_sb[:, 4 * g + h, nsl],
                                rhs=qq_sb[:, 4 * g + h, msl],
                                start=True,
                                stop=True,
                            )
                        for j in range(2):
                            nc.scalar.activation(
                                ex[:w, ds(2 * j, 2), msl], sts[j][:w, :, 0:MC], AF.Exp
                            )

                    prev_ex = None
                    prev_w = None
                    for nt in range(15):
                        ex = None
                        if nt < 14:
                            ex = rot.tile([128, 4, M], F32, tag="expst")
                            emit_st(nt, 0, ex)
                        # interleaved filler between the two S^T chunks covers
                        # the st-tile reuse window (exp must drain the tile)
                        if nt < 14:
                            emit_vt(psB, nt, g)
                        if g == 0 and nt == 1:
                            qrg1 = stage.tile([128, M], F32R, tag="qrg", name="qrg1")
                            qeg1 = stage.tile([128, M], F32R, tag="qeg", name="qeg1")
                            krg1 = stage.tile([128, N], F32R, tag="krg", name="krg1")
                            keg1 = stage.tile([128, N], F32R, tag="keg", name="keg1")
                        if g == 0 and nt in (1, 2):
                            emit_q_mc(psB, "aux", 1, nt - 1, qrg1, qeg1)
                        if g == 0 and nt == 2:
                            emit_q_stacks(1, qrg1, qeg1)
                        if g == 0 and nt in (3, 4, 5, 6):
                            emit_k_chunk(psB, "aux", 1, nt - 3, krg1, keg1)
                        if g == 0 and nt == 6:
                            emit_k_stacks(1, 0, N, krg1, keg1, dual=True)
                        if nt < 14:
                            emit_st(nt, 1, ex)
                        # A@V runs one nt behind the S^T/exp pipeline
                        if nt >= 1:
                            pw = prev_w
                            for h in range(4):
                                for mt in range(7):
                                    mw = MT_SIZES[mt]
                                    nc.tensor.matmul(
                                        ot_ps[h // 2][:mw, mt, h % 2, 0:33],
                                        lhsT=prev_ex[:pw, h, ds(mt * 128, mw)],
                                        rhs=vt_sb[:pw, nt - 1, 4 * g + h, :],
                                        start=(nt == 1 and mt == 0 and h % 2 == 0),
                                        stop=(nt == 14 and mt == 6 and h % 2 == 1),
                                    )
                        if nt < 14:
                            prev_ex = ex
                            prev_w = NT_SIZES[nt]
                    # normalize O^T[:, 0:32] / O^T[:, 32] (batched reciprocals,
                    # then per-partition scalar multiplies), and transpose this
                    # group's O^T back to O — per m-tile so the projection can
                    # start as soon as its columns are ready. Group 0's scalar
                    # multiplies run on DVE (ACT is busy with group 1's exps);
                    # group 1's run on the by-then-idle ScalarE.
                    rs14s = []
                    for j in range(2):
                        rs14 = rot.tile([128, 7, 2], F32, tag="rs", name="rs14")
                        nc.vector.reciprocal(rs14, ot_ps[j][:, :, :, 32:33])
                        rs14s.append(rs14)
                    if g == 1:
                        # keep the PE p-state warm through the normalize bubble
                        # so the transposes + projection run at full speed (the
                        # ex operand pins these behind the last exp)
                        for i in range(8):
                            wu2 = psB.tile([128, 128], F32, tag="aux", name="wu2", bufs=2)
                            nc.tensor.matmul(
                                wu2, lhsT=wu_in, rhs=prev_ex[:, 0, 0:128], start=True, stop=True
                            )
                    for mt in range(7):
                        mw = MT_SIZES[mt]
                        for j in range(2):
                            for hh in range(2):
                                h = 2 * j + hh
                                if g == 0 or j == 0:
                                    nc.vector.tensor_scalar_mul(
                                        on_t_sb[:mw, mt, ds(32 * (4 * g + h), 32)],
                                        ot_ps[j][:mw, mt, hh, 0:32],
                                        rs14s[j][:mw, mt, hh : hh + 1],
                                    )
                                else:
                                    nc.scalar.activation(
                                        on_t_sb[:mw, mt, ds(32 * (4 * g + h), 32)],
                                        ot_ps[j][:mw, mt, hh, 0:32],
                                        AF.Copy,
                                        scale=rs14s[j][:mw, mt, hh : hh + 1],
                                    )
                        tp = psB.tile([128, 128], F32, tag="aux", name="tp", bufs=2)
                        nc.tensor.transpose(
                            tp[:, :mw],
                            on_t_sb[:mw, mt, ds(g * 128, 128)],
                            ident[:mw, :mw],
                        )
                        nc.vector.tensor_copy(on_sb[:, g, ds(mt * 128, mw)], tp[:, :mw])

                # ---- projection (PSUM slots reuse the st tag) ----
                for mc in range(N_MC):
                    for ot in range(2):
                        yp = psB.tile([128, MC], F32, tag="st0", name="yp")
                        for g2 in range(2):
                            nc.tensor.matmul(
                                yp,
                                lhsT=wp_sb[:, g2, ot, :],
                                rhs=on_sb[:, g2, ds(mc * MC, MC)],
                                start=(g2 == 0),
                                stop=(g2 == 1),
                            )
                        nc.scalar.activation(
                            y_sb[:, ot, ds(mc * MC, MC)], yp, AF.Identity,
                            bias=b_sb[:, ot, :], scale=1.0,
                        )
                        nc.sync.dma_start(
                            out=y_d.rearrange("(ot p) m -> ot p m", p=128)[ot, :, ds(mc * MC, MC)],
                            in_=y_sb[:, ot, ds(mc * MC, MC)],
                        )

    nc.compile()
    return nc


_NC = None


def _get_nc():
    global _NC
    if _NC is None:
        _NC = build_program()
    return _NC


def make_in_maps(x, w_qkv, w_proj, b_proj):
    x = np.asarray(x, np.float32)
    w_qkv = np.asarray(w_qkv, np.float32)
    w_proj = np.asarray(w_proj, np.float32)
    b_proj = np.asarray(b_proj, np.float32)
    P = x.shape[0]
    xf = np.ascontiguousarray(x.reshape(P, C, N))
    wqT = np.ascontiguousarray((w_qkv[0:C] * SCALE).T)
    wkT = np.ascontiguousarray(w_qkv[C : 2 * C].T)
    wvT = np.ascontiguousarray(w_qkv[2 * C : 3 * C].T)
    wpT = np.ascontiguousarray(w_proj.T)
    bias = np.ascontiguousarray(b_proj.reshape(C, 1))
    in_maps = []
    for core in range(N_CORES):
        p, mh = divmod(core, 2)
        if mh == 0:
            xp = xf[p]
        else:
            # rotate the n axis so this core's query half comes first
            xp = np.concatenate([xf[p][:, M:], xf[p][:, :M]], axis=1)
        in_maps.append(
            {
                "xf": np.ascontiguousarray(xp),
                "wqT": wqT,
                "wkT": wkT,
                "wvT": wvT,
                "wpT": wpT,
                "bias": bias,
            }
        )
    return in_maps


def assemble_output(per_core_y, x_shape):
    P, B, _, H, W, D = x_shape
    y = np.empty((P, C, N), np.float32)
    for core in range(N_CORES):
        p, mh = divmod(core, 2)
        y[p][:, mh * M : (mh + 1) * M] = per_core_y[core]
    return y.reshape(P, B, C, H, W, D)


def kernel(x, w_qkv, w_proj, b_proj):
    nc = _get_nc()
    in_maps = make_in_maps(x, w_qkv, w_proj, b_proj)
    res = run_bass_kernel_spmd(nc, in_maps, core_ids=list(range(N_CORES)))
    return assemble_output([res.results[c]["y"] for c in range(N_CORES)], x.shape)
